# revision 7
# baseline (speedup 1.0000x reference)
"""AttentionBlock (GroupNorm -> qkv conv1x1 -> 8-head attention -> proj -> residual)
on 8 trn2 NeuronCores.

Sharding: core c handles batch b = c//2 and head-half h = c%2 (4 of 8 heads).
Each core computes GroupNorm(x[b]) (duplicated over the 2 cores of a batch),
the qkv rows for its 4 heads, attention for those heads, and a partial
proj_out contribution (proj_w restricted to its heads' input channels).
Host combines: y[b] = x[b] + proj_b + partial[2b] + partial[2b+1].

Attention layout (per head, head_dim D=64, L=2048):
  scores^T = k^T q computed as matmul(lhsT=k_chunk, rhs=q)  -> [keys, q] in PSUM
  E = exp(0.125 * scores^T)  (no max subtraction: scores ~ N(0,1), max ~ 6)
  out/sumexp = matmul(lhsT=[v^T | ones], rhs=E) accumulated over key chunks:
  rows 0-63 = unnormalized out, row 64 = sum of exp. Divide on DVE.
"""

import numpy as np
import ml_dtypes

import concourse.bass as bass
import concourse.tile as tile
from concourse import bacc, mybir
from concourse.bass_utils import run_bass_kernel_spmd

BF16 = mybir.dt.bfloat16
F32 = mybir.dt.float32
AF = mybir.ActivationFunctionType
OP = mybir.AluOpType

B, C, L = 4, 512, 2048
H, D = 8, 64
G = 32  # groupnorm groups
EPS = 1e-5
N_CORES = 8
CLOC = 256  # channels of the 4 local heads


def _bcast_partitions(ap, n):
    # Re-read the same single-partition row n times: partition dim stays
    # count-1, an extra 0-step free dim repeats the row for the n
    # destination partitions.
    return bass.AP(tensor=ap.tensor, offset=ap.offset,
                   ap=[list(ap.ap[0]), [0, n]] + [list(d) for d in ap.ap[1:]])


def _build_program(dbg=False):
    nc = bacc.Bacc("TRN2", target_bir_lowering=False, debug=False,
                   num_devices=N_CORES)
    if dbg:
        dbg_h = nc.dram_tensor("dbg_h", [4, 128, L], BF16, kind="ExternalOutput")
        dbg_q = nc.dram_tensor("dbg_q", [2, 128, L], BF16, kind="ExternalOutput")
        dbg_k = nc.dram_tensor("dbg_k", [2, 128, L], BF16, kind="ExternalOutput")
        dbg_v = nc.dram_tensor("dbg_v", [2, 128, L], BF16, kind="ExternalOutput")
        dbg_vt = nc.dram_tensor("dbg_vt", [2, 16, 128, 130], BF16,
                                kind="ExternalOutput")
        dbg_oh = nc.dram_tensor("dbg_oh", [4, 64, L], BF16,
                                kind="ExternalOutput")
        dbg_e = nc.dram_tensor("dbg_e", [16, 128, 1024], BF16,
                               kind="ExternalOutput")

    x_d = nc.dram_tensor("x", [C, L], F32, kind="ExternalInput")
    wqkvT_d = nc.dram_tensor("wqkvT", [4, 128, 768], BF16, kind="ExternalInput")
    bqkv_d = nc.dram_tensor("bqkv", [128, 6], F32, kind="ExternalInput")
    gnw_d = nc.dram_tensor("gnw", [128, 4], F32, kind="ExternalInput")
    gnb_d = nc.dram_tensor("gnb", [128, 4], F32, kind="ExternalInput")
    ind_d = nc.dram_tensor("ind", [128, 8], F32, kind="ExternalInput")
    indT_d = nc.dram_tensor("indT", [8, 128], F32, kind="ExternalInput")
    projT_d = nc.dram_tensor("projT", [4, 64, 512], BF16, kind="ExternalInput")
    out_d = nc.dram_tensor("out", [C, L], F32, kind="ExternalOutput")

    with tile.TileContext(nc) as tc:
        with (
            tc.tile_pool(name="psum", bufs=2, space="PSUM") as psum,
            tc.tile_pool(name="consts", bufs=1) as consts,
            tc.tile_pool(name="sb", bufs=2) as sb,
        ):
            # ---- constants / weights in ----
            zero_c = consts.tile([128, 1], F32)
            nc.vector.memset(zero_c[:], 0.0)
            nc.const_aps.aps[(F32, 0.0)] = zero_c[:]
            eps_t = consts.tile([8, 1], F32)
            nc.vector.memset(eps_t[:], EPS)
            bq_sb = consts.tile([128, 6], F32)
            nc.sync.dma_start(out=bq_sb[:], in_=bqkv_d.ap())
            gnw_sb = consts.tile([128, 4], F32)
            nc.sync.dma_start(out=gnw_sb[:], in_=gnw_d.ap())
            gnb_sb = consts.tile([128, 4], F32)
            nc.sync.dma_start(out=gnb_sb[:], in_=gnb_d.ap())
            ind_sb = consts.tile([128, 8], F32)
            nc.sync.dma_start(out=ind_sb[:], in_=ind_d.ap())
            indT_sb = consts.tile([8, 128], F32)
            nc.sync.dma_start(out=indT_sb[:], in_=indT_d.ap())
            wT_sb = []
            for kc in range(4):
                wt = sb.tile([128, 768], BF16, tag="wT", bufs=4, name=f"wT{kc}")
                nc.sync.dma_start(out=wt[:], in_=wqkvT_d.ap()[kc])
                wT_sb.append(wt)
            projT_sb = []
            for lh in range(4):
                pt = sb.tile([64, 512], BF16, tag="projT", bufs=4, name=f"pT{lh}")
                nc.sync.dma_start(out=pt[:], in_=projT_d.ap()[lh])
                projT_sb.append(pt)

            # ---- GroupNorm ----
            xa = x_d.ap()
            x_sb = []
            statsall = consts.tile([128, 8], F32)  # per chunk: [mean_p, var_p]
            for c in range(4):
                xc = sb.tile([128, L], F32, tag="x", bufs=4, name=f"x{c}")
                nc.sync.dma_start(out=xc[:], in_=xa[c * 128:(c + 1) * 128, :])
                x_sb.append(xc)
                stats6 = sb.tile([128, 4, 6], F32, tag="bnst", bufs=2,
                                 name=f"bnst{c}")
                for s in range(4):
                    nc.vector.bn_stats(out=stats6[:, s, :],
                                       in_=xc[:, s * 512:(s + 1) * 512])
                nc.vector.bn_aggr(out=statsall[:, 2 * c:2 * c + 2], in_=stats6[:])

            # per-partition E[x^2] = var + mean^2 (in place in the var slots)
            msr = statsall.rearrange("p (c two) -> p c two", two=2)
            sq = consts.tile([128, 4], F32)
            nc.vector.tensor_mul(out=sq[:], in0=msr[:, :, 0], in1=msr[:, :, 0])
            nc.vector.tensor_add(out=msr[:, :, 1], in0=msr[:, :, 1], in1=sq[:])

            # group sums over the 16 partitions of each group
            gstats = psum.tile([8, 8], F32, tag="av")
            nc.tensor.matmul(gstats[:], lhsT=ind_sb[:], rhs=statsall[:])
            gp = consts.tile([8, 8], F32)
            nc.vector.tensor_scalar_mul(out=gp[:], in0=gstats[:], scalar1=1.0 / 16.0)
            gpr = gp.rearrange("p (c two) -> p c two", two=2)
            var4 = consts.tile([8, 4], F32)
            nc.vector.tensor_mul(out=var4[:], in0=gpr[:, :, 0], in1=gpr[:, :, 0])
            # var = E[x^2] - mu^2
            nc.vector.scalar_tensor_tensor(out=var4[:], in0=var4[:], scalar=-1.0,
                                           in1=gpr[:, :, 1], op0=OP.mult,
                                           op1=OP.add)
            # rstd = exp(-0.5 * ln(var + eps))
            lnv = consts.tile([8, 4], F32)
            nc.scalar.activation(out=lnv[:], in_=var4[:], func=AF.Ln,
                                 bias=eps_t[:])
            rstd4 = consts.tile([8, 4], F32)
            nc.scalar.activation(out=rstd4[:], in_=lnv[:], func=AF.Exp, scale=-0.5)
            brd = consts.tile([8, 8], F32)
            brr = brd.rearrange("p (c two) -> p c two", two=2)
            nc.vector.tensor_copy(out=brr[:, :, 0], in_=rstd4[:])
            nc.vector.tensor_mul(out=brr[:, :, 1], in0=gpr[:, :, 0], in1=rstd4[:])
            bcast = psum.tile([128, 8], F32, tag="av")
            nc.tensor.matmul(bcast[:], lhsT=indT_sb[:], rhs=brd[:])
            bcr = bcast.rearrange("p (c two) -> p c two", two=2)

            h_sb = []
            for c in range(4):
                wsc = sb.tile([128, 1], F32, tag="wsc", bufs=4, name=f"wsc{c}")
                nc.vector.tensor_mul(out=wsc[:], in0=bcr[:, c, 0:1],
                                     in1=gnw_sb[:, c:c + 1])
                tmp = sb.tile([128, 1], F32, tag="wtmp", bufs=4, name=f"wtmp{c}")
                nc.vector.tensor_mul(out=tmp[:], in0=bcr[:, c, 1:2],
                                     in1=gnw_sb[:, c:c + 1])
                wbi = sb.tile([128, 1], F32, tag="wbi", bufs=4, name=f"wbi{c}")
                nc.vector.tensor_tensor(out=wbi[:], in0=gnb_sb[:, c:c + 1],
                                        in1=tmp[:], op=OP.subtract)
                hc = sb.tile([128, L], BF16, tag="h", bufs=4, name=f"h{c}")
                nc.vector.tensor_scalar(out=hc[:], in0=x_sb[c][:], scalar1=wsc[:],
                                        scalar2=wbi[:], op0=OP.mult, op1=OP.add)
                h_sb.append(hc)
                if dbg:
                    nc.sync.dma_start(out=dbg_h.ap()[c], in_=hc[:])

            # ---- QKV: [768,512] @ h + b, m-chunks of 128 output rows ----
            qp = [sb.tile([128, L], BF16, tag="qp", bufs=2, name=f"qp{p}")
                  for p in range(2)]
            kp = [sb.tile([128, L], BF16, tag="kp", bufs=2, name=f"kp{p}")
                  for p in range(2)]
            vp = [sb.tile([128, L], BF16, tag="vp", bufs=2, name=f"vp{p}")
                  for p in range(2)]
            dest = {0: qp[0], 1: qp[1], 2: kp[0], 3: kp[1], 4: vp[0], 5: vp[1]}
            vt = {0: [], 1: []}  # per pair, per kc: [vT_A | 1 | vT_B | 1]
            for m in (4, 2, 0, 5, 3, 1):
                for n in range(2):
                    ps = psum.tile([128, 1024], F32, tag="st", name=f"qkv{m}{n}")
                    for kc in range(4):
                        for s in range(2):
                            o = n * 1024 + s * 512
                            nc.tensor.matmul(
                                ps[:, s * 512:(s + 1) * 512],
                                lhsT=wT_sb[kc][:, m * 128:(m + 1) * 128],
                                rhs=h_sb[kc][:, o:o + 512],
                                start=(kc == 0), stop=(kc == 3))
                    nc.vector.tensor_scalar_add(
                        out=dest[m][:, n * 1024:(n + 1) * 1024], in0=ps[:],
                        scalar1=bq_sb[:, m:m + 1])
                if dbg:
                    dd = {0: dbg_q, 1: dbg_q, 2: dbg_k, 3: dbg_k,
                          4: dbg_v, 5: dbg_v}[m]
                    nc.sync.dma_start(out=dd.ap()[m % 2], in_=dest[m][:])
                if m >= 4:  # v chunk done -> transpose its 16 key-chunks
                    pair = m - 4
                    for kc in range(16):
                        pr = []
                        for hd in range(2):
                            t = sb.tile([128, 65], BF16, tag="vt", bufs=64,
                                        name=f"vt{pair}_{kc}_{hd}")
                            nc.vector.memset(t[:, 64:65], 1.0)
                            nc.sync.dma_start_transpose(
                                out=t[:, 0:64],
                                in_=vp[pair][hd * 64:(hd + 1) * 64,
                                             kc * 128:(kc + 1) * 128])
                            pr.append(t)
                            if dbg:
                                nc.sync.dma_start(
                                    out=dbg_vt.ap()[pair, kc, :, hd * 65:hd * 65 + 65],
                                    in_=t[:])
                        vt[pair].append(pr)

            # ---- attention ----
            oh = []
            for lh in range(4):
                t = sb.tile([64, L], BF16, tag="oh", bufs=4, name=f"oh{lh}")
                oh.append(t)
            for pair in range(2):
                for hd in range(2):
                    lh = pair * 2 + hd
                    r0 = hd * 64
                    for qb in range(2):
                        av = psum.tile([65, 1024], F32, tag="av",
                                       name=f"av{lh}{qb}")
                        for kc in range(16):
                            st = psum.tile([128, 1024], F32, tag="st",
                                           name=f"st{lh}{qb}{kc}")
                            for s in range(2):
                                o = qb * 1024 + s * 512
                                nc.tensor.matmul(
                                    st[:, s * 512:(s + 1) * 512],
                                    lhsT=kp[pair][r0:r0 + 64,
                                                  kc * 128:(kc + 1) * 128],
                                    rhs=qp[pair][r0:r0 + 64, o:o + 512])
                            ex = sb.tile([128, 1024], BF16, tag="E", bufs=3,
                                         name=f"E{lh}{qb}{kc}")
                            nc.scalar.activation(out=ex[:], in_=st[:],
                                                 func=AF.Exp, scale=0.125)
                            if dbg and lh == 0 and qb == 0:
                                nc.sync.dma_start(out=dbg_e.ap()[kc], in_=ex[:])
                            lw = vt[pair][kc][hd][:, 0:65]
                            for s in range(2):
                                nc.tensor.matmul(
                                    av[:, s * 512:(s + 1) * 512], lhsT=lw,
                                    rhs=ex[:, s * 512:(s + 1) * 512],
                                    start=(kc == 0), stop=(kc == 15),
                                    skip_group_check=True)
                        rc = sb.tile([65, 1024], F32, tag="rc", bufs=2,
                                     name=f"rc{lh}{qb}")
                        nc.vector.reciprocal(out=rc[64:65, :], in_=av[64:65, :])
                        rcb = sb.tile([64, 1024], F32, tag="rcb", bufs=2,
                                      name=f"rcb{lh}{qb}")
                        nc.gpsimd.dma_start(out=rcb[:],
                                            in_=_bcast_partitions(rc[64:65, :], 64))
                        nc.vector.tensor_tensor(
                            out=oh[lh][:, qb * 1024:(qb + 1) * 1024],
                            in0=av[0:64, :], in1=rcb[:], op=OP.mult)

            if dbg:
                for lh in range(4):
                    nc.sync.dma_start(out=dbg_oh.ap()[lh], in_=oh[lh][:])
            # ---- partial proj: [512, 256] @ oh ----
            oa = out_d.ap()
            for m in range(4):
                for ns in range(4):
                    pp = psum.tile([128, 512], F32, tag="st", name=f"pp{m}{ns}")
                    for lh in range(4):
                        nc.tensor.matmul(
                            pp[:], lhsT=projT_sb[lh][:, m * 128:(m + 1) * 128],
                            rhs=oh[lh][:, ns * 512:(ns + 1) * 512],
                            start=(lh == 0), stop=(lh == 3))
                    of = sb.tile([128, 512], F32, tag="of", bufs=3,
                                 name=f"of{m}{ns}")
                    nc.vector.tensor_copy(out=of[:], in_=pp[:])
                    nc.sync.dma_start(
                        out=oa[m * 128:(m + 1) * 128, ns * 512:(ns + 1) * 512],
                        in_=of[:])

    nc.compile()
    return nc


_NC = None


def _get_nc():
    global _NC
    if _NC is None:
        _NC = _build_program()
    return _NC


def _make_in_maps(x, norm_w, norm_b, qkv_w, qkv_b, proj_w):
    bf = ml_dtypes.bfloat16
    gnw = np.ascontiguousarray(norm_w.reshape(4, 128).T, np.float32)
    gnb = np.ascontiguousarray(norm_b.reshape(4, 128).T, np.float32)
    ind = np.zeros((128, 8), np.float32)
    ind[np.arange(128), np.arange(128) // 16] = 1.0
    indT = np.ascontiguousarray(ind.T)
    in_maps = []
    for core in range(N_CORES):
        b, half = core // 2, core % 2
        rows = slice(half * CLOC, (half + 1) * CLOC)
        w_loc = np.concatenate(
            [qkv_w[rows], qkv_w[C:][rows], qkv_w[2 * C:][rows]], axis=0)
        wT = np.ascontiguousarray(w_loc.T, np.float32).reshape(4, 128, 768)
        b_loc = np.concatenate(
            [qkv_b[rows], qkv_b[C:][rows], qkv_b[2 * C:][rows]])
        bq = np.ascontiguousarray(b_loc.reshape(6, 128).T, np.float32)
        pT = np.stack([
            np.ascontiguousarray(
                proj_w[:, half * CLOC + lh * 64: half * CLOC + (lh + 1) * 64].T)
            for lh in range(4)]).astype(np.float32)
        in_maps.append({
            "x": np.ascontiguousarray(x[b], np.float32),
            "wqkvT": wT.astype(bf),
            "bqkv": bq,
            "gnw": gnw,
            "gnb": gnb,
            "ind": ind,
            "indT": indT,
            "projT": pT.astype(bf),
        })
    return in_maps


def run_cores(in_maps):
    nc = _get_nc()
    res = run_bass_kernel_spmd(nc, in_maps, core_ids=list(range(N_CORES)))
    return res


def kernel(x, norm_w, norm_b, qkv_w, qkv_b, proj_w, proj_b):
    x = np.asarray(x, np.float32)
    in_maps = _make_in_maps(x, np.asarray(norm_w, np.float32),
                            np.asarray(norm_b, np.float32),
                            np.asarray(qkv_w, np.float32),
                            np.asarray(qkv_b, np.float32),
                            np.asarray(proj_w, np.float32))
    res = run_cores(in_maps)
    y = np.empty((B, C, L), np.float32)
    pb = np.asarray(proj_b, np.float32)[:, None]
    for b in range(B):
        y[b] = (x[b] + pb + res.results[2 * b]["out"]
                + res.results[2 * b + 1]["out"])
    return y
